# revision 46
# baseline (speedup 1.0000x reference)
"""GAT-style attention kernel for Trainium2, 8 NeuronCores.

Reference computation (N=M=8192, D=256, f32):
    e1 = input1 @ a1; e2 = (input2 @ a2).T
    e  = leaky_relu(e1 + e2, 0.2)
    att = softmax(where(adj>0, e, -9e15), axis=1)
    att = att * adj.sum(1, keepdims=True)
    att = att*0.5 + adj*0.5
    out = att @ input2

Device math per row i (w_ij = exp(leaky_relu(e1_i + e2_j))):
    denom_i = sum_j adj_ij w_ij ; deg_i = sum_j adj_ij ; delta_i = deg_i/denom_i
    out_i = 0.5 * [ (delta_i * (adj.w) + adj) @ input2 ]_i

Sharding: rows of N across 8 cores (1024 each); input2/a1/a2 replicated.

Column-panelized pipeline: each 128-row block is processed in NP=4 panels
of 2048 columns so DVE/Pool/ACT/PE overlap within a block:
    adjb = bf16(adj panel)                 [SWDGE cast-DMA, per panel]
    mt   = adjb - 1, accum -> deg[q]       [DVE TS, 4x mode]
    mt  += e2b' (e2/BIG, f16)              [TT, per-panel DVE/Pool]
    lr   = prelu(BIG*mt + e1_i, 0.2)       [per-panel ACT or DVE TS/TS/TTmax]
    num  = exp(lr), accum -> den[q]        [ACT, bf16 out]
    satt = num * delta_i                   [DVE TS]
    satt += adjb                           [TT, per-panel DVE/Pool]
    sattT: PE transpose + PSUM->SBUF copy, or per-panel SBUF->SBUF DMA
           transpose (per-block route)
    acc += sattT_chunk.T @ input2_bf16_chunk   [PE matmul]
    out = 0.5 * acc                        [DVE]
Setup is panelized too: input2 cast-DMA quarter -> 16 AMRs -> e2 scatter ->
e2b broadcast, per panel, so block 0 starts before setup fully drains.
"""

import os
import numpy as np

import concourse.bass as bass
import concourse.bacc as bacc
import concourse.tile as tile
from concourse import mybir
from concourse.bass_utils import run_bass_kernel_spmd

N, M, D = 8192, 8192, 256
NCORES = 8
ROWS = N // NCORES
P = 128
NBLK = ROWS // P  # 8
NCHUNK = M // P  # 64
BIG = 150.0
SLOPE = 0.2
GRP = 8     # transpose chunks per PSUM staging bank (1024 cols)
NP = int(os.environ.get("K_NP", "4"))  # column panels per block
PW = M // NP  # 2048
CPP = PW // P  # chunks per panel = 16

F32 = mybir.dt.float32
BF16 = mybir.dt.bfloat16
F16 = mybir.dt.float16


def _intset(env, default):
    return set(int(x) for x in os.environ.get(env, default).split(",") if x != "")


# ---- tuning knobs ----------------------------------------------------
# blocks whose sattT comes from SBUF->SBUF DMA transpose (others: PE+copy)
DMA_ROUTE = _intset("K_DMAROUTE", "0")
# per-panel engine for mt += e2b: 'v'=DVE, 'p'=Pool
TT_ENG = os.environ.get("K_TTENG", "vvvv")
# per-panel engine for prelu: 'a'=ACT, 'v'=DVE
PRE_ENG = os.environ.get("K_PREENG", "avaa")
# per-panel engine for satt += adjb: 'v'=DVE, 'p'=Pool
ST_ENG = os.environ.get("K_STENG", "vvvv")
# PSUM->SBUF copy engine per group index (PE route): 'v'=DVE, 's'=ACT
COPY_ENG = os.environ.get("K_COPYENG", "vvvs")

LAST_EXEC_NS = None
_CACHED = None


def _build_kernel():
    nc = bacc.Bacc("TRN2", target_bir_lowering=False, debug=False)

    inp1 = nc.dram_tensor("input1", [ROWS, D], F32, kind="ExternalInput").ap()
    inp2 = nc.dram_tensor("input2", [M, D], F32, kind="ExternalInput").ap()
    adj = nc.dram_tensor("adj", [ROWS, M], F32, kind="ExternalInput").ap()
    a1b = nc.dram_tensor("a1b", [P, D], F32, kind="ExternalInput").ap()
    a2b = nc.dram_tensor("a2b", [P, D], F32, kind="ExternalInput").ap()
    identd = nc.dram_tensor("identd", [P, P], BF16, kind="ExternalInput").ap()
    out = nc.dram_tensor("out", [ROWS, D], F32, kind="ExternalOutput").ap()

    # DRAM bounce for flattening e2 (computed column-wise) into row order
    e2d = nc.dram_tensor("e2d", [1, M], F32).ap()

    AL = mybir.AluOpType

    with tile.TileContext(nc) as tc:
        with (
            tc.tile_pool(name="persist", bufs=1) as persist,
            tc.tile_pool(name="small", bufs=4) as small,
            tc.tile_pool(name="adjp", bufs=3) as adjp,
            tc.tile_pool(name="mtp", bufs=2) as mtp,
            tc.tile_pool(name="nump", bufs=2) as nump,
            tc.tile_pool(name="attp", bufs=2) as attp,
            tc.tile_pool(name="tatp", bufs=2) as tatp,
            tc.tile_pool(name="outp", bufs=2) as outp,
            tc.tile_pool(name="psA", bufs=3, space="PSUM") as psA,
            tc.tile_pool(name="psO", bufs=2, space="PSUM") as psO,
        ):
            # ---------------- setup (panelized) ----------------
            ident = persist.tile([P, P], BF16)
            nc.sync.dma_start(ident[:], identd[:])
            a1t = persist.tile([P, D], F32)
            nc.sync.dma_start(a1t[:], a1b[:])
            a2t = persist.tile([P, D], F32)
            nc.sync.dma_start(a2t[:], a2b[:])

            inp2b = persist.tile([P, NCHUNK * D], BF16)
            e2col = persist.tile([P, NCHUNK], F32)
            e1col = persist.tile([P, NBLK], F32)
            e2b = persist.tile([P, M], F16)
            e2d_scat = e2d.rearrange("one (t p) -> one p t", p=P, t=NCHUNK)

            adjb_t = {}
            for b0 in (0, 1):
                adjb_t[b0] = adjp.tile([P, M], BF16, name="adjb", tag="adjb")
            itile = mtp.tile([P, NBLK * D], F32, tag="mt")

            # per-panel setup chain, q0 first so front(0,0) starts early:
            # inp2 quarter -> 16 AMRs -> e2 scatter -> e2b broadcast, with
            # adjb panels interleaved behind each quarter
            def _amrs(q):
                for k in range(CPP):
                    t = q * CPP + k
                    scr = small.tile([P, 1], F32, tag="amrdummy")
                    nc.vector.affine_mul_reduce(
                        out=scr[:].broadcast_to([P, D]),
                        accum_out=e2col[:, t : t + 1],
                        in0=inp2b[:, t * D : (t + 1) * D],
                        in1=a2t[:],
                        scale=1.0 / BIG,
                        bias=0.0,
                    )
                nc.vector.tensor_scalar(
                    inp2b[:, q * CPP * D : (q + 1) * CPP * D],
                    inp2b[:, q * CPP * D : (q + 1) * CPP * D],
                    0.5, None, AL.mult,
                )

            def _e2chain(q):
                # amrs -> scatter -> broadcast; always emitted as a unit so
                # the scatter never precedes its producer AMRs
                _amrs(q)
                nc.sync.dma_start(
                    e2d_scat[0][:, q * CPP : (q + 1) * CPP],
                    e2col[:, q * CPP : (q + 1) * CPP],
                )
                nc.gpsimd.dma_start(
                    e2b[:, q * PW : (q + 1) * PW],
                    e2d[:, q * PW : (q + 1) * PW].broadcast_to([P, PW]),
                )

            for q in range(NP):
                src = inp2[q * CPP * P : (q + 1) * CPP * P, :]
                src = src.rearrange("(c p) d -> p c d", p=P, c=CPP)
                dst = inp2b[:, q * CPP * D : (q + 1) * CPP * D]
                dst = dst.rearrange("p (c d) -> p c d", c=CPP, d=D)
                nc.gpsimd.dma_start(dst, src)
                if q == 0:
                    nc.gpsimd.dma_start(
                        adjb_t[0][:, 0:PW], adj[0:P, 0:PW]
                    )
                _e2chain(q)
                if q > 0:
                    nc.gpsimd.dma_start(
                        adjb_t[0][:, q * PW : (q + 1) * PW],
                        adj[0:P, q * PW : (q + 1) * PW],
                    )
                if q == 0:
                    src1 = inp1.rearrange("(c p) d -> p c d", p=P, c=NBLK)
                    nc.sync.dma_start(
                        itile[:].rearrange("p (c d) -> p c d", c=NBLK, d=D), src1
                    )
                    for b1 in range(NBLK):
                        scr = small.tile([P, 1], F32, tag="amrdummy")
                        nc.vector.affine_mul_reduce(
                            out=scr[:].broadcast_to([P, D]),
                            accum_out=e1col[:, b1 : b1 + 1],
                            in0=itile[:, b1 * D : (b1 + 1) * D],
                            in1=a1t[:],
                            scale=1.0,
                            bias=0.0,
                        )
                nc.gpsimd.dma_start(
                    adjb_t[1][:, q * PW : (q + 1) * PW],
                    adj[P : 2 * P, q * PW : (q + 1) * PW],
                )

            def _setup_hook(q):
                pass

            # ---------------- main loop (software-pipelined, 1-block skew) ----
            num_t, deg_t, den_t = {}, {}, {}

            front_state = {}

            def emit_front_panel(b, q):
                if q == 0:
                    fmt = mtp.tile([P, M], F16, name="fmt", tag="mt")
                    front_state[b] = (fmt, fmt)
                    deg4_ = small.tile([P, NP], F32, name="deg4_", tag="deg")
                    den4_ = small.tile([P, NP], F32, name="den4_", tag="den")
                    num_ = nump.tile([P, M], BF16, name="num_", tag="num")
                    deg_t[b], den_t[b], num_t[b] = deg4_, den4_, num_
                mt, lrt = front_state[b]
                deg4, den4, num = deg_t[b], den_t[b], num_t[b]
                if True:
                    qs = slice(q * PW, (q + 1) * PW)
                    nc.vector.tensor_scalar(
                        mt[:, qs], adjb_t[b][:, qs], -1.0, 0.0, AL.add, AL.add,
                        accum_out=deg4[:, q : q + 1],
                    )
                    if TT_ENG[q] == "p":
                        nc.gpsimd.tensor_tensor(mt[:, qs], mt[:, qs], e2b[:, qs], AL.add)
                    else:
                        nc.vector.tensor_tensor(mt[:, qs], mt[:, qs], e2b[:, qs], AL.add)
                    lr = mt[:, qs]  # prelu in place: mt panel dead after
                    nc.scalar.activation(
                        lr, mt[:, qs], mybir.ActivationFunctionType.Prelu,
                        bias=e1col[:, b : b + 1], scale=BIG, alpha=SLOPE,
                    )
                    nc.scalar.activation(
                        num[:, qs], lr, mybir.ActivationFunctionType.Exp,
                        accum_out=den4[:, q : q + 1],
                    )

            back_state = {}

            def emit_back_head(b):
                den = small.tile([P, 1], F32, tag="denS")
                nc.vector.tensor_reduce(den[:], den_t[b][:], mybir.AxisListType.X, AL.add)
                rec = small.tile([P, 1], F32, tag="rec")
                nc.vector.reciprocal(rec[:], den[:])
                deg = small.tile([P, 1], F32, tag="degS")
                nc.vector.tensor_reduce(deg[:], deg_t[b][:], mybir.AxisListType.X, AL.add)
                degm = small.tile([P, 1], F32, tag="degm")
                nc.vector.tensor_scalar(degm[:], deg[:], float(M), None, AL.add)
                delta = small.tile([P, 1], F32, tag="delta")
                nc.vector.tensor_tensor(delta[:], degm[:], rec[:], AL.mult)
                satt = num_t[b]  # satt overwrites num in place (dead after)
                adjb = adjb_t.pop(b)
                acc = psO.tile([P, D], F32)
                back_state[b] = (satt, adjb, delta, acc)

            def emit_back_panel(b, q):
                satt, adjb, delta, acc = back_state[b]
                GW = GRP * P  # 1024 columns per group
                if b in DMA_ROUTE:
                    qs = slice(q * PW, (q + 1) * PW)
                    nc.vector.tensor_scalar(
                        satt[:, qs], num_t[b][:, qs], delta[:], None, AL.mult
                    )
                    if ST_ENG[q] == "p":
                        nc.gpsimd.tensor_tensor(
                            satt[:, qs], satt[:, qs], adjb[:, qs], AL.add
                        )
                    else:
                        nc.vector.tensor_tensor(
                            satt[:, qs], satt[:, qs], adjb[:, qs], AL.add
                        )
                    if PANELT:
                        tat = tatp.tile([P, CPP, P], BF16, tag="tat")
                        nc.sync.dma_start(
                            tat[:],
                            satt[:, qs].rearrange("p (c w) -> p c w", c=CPP, w=P),
                            transpose=True,
                        )
                        for k in range(CPP):
                            c = q * CPP + k
                            nc.tensor.matmul(
                                acc[:],
                                tat[:, k, :],
                                inp2b[:, c * D : (c + 1) * D],
                                start=(c == 0), stop=(c == NCHUNK - 1),
                            )
                    elif q == NP - 1:
                        # one whole-block transpose; all 64 matmuls become
                        # ready together, keeping the PE streak unbroken
                        tat = tatp.tile([P, NCHUNK, P], BF16, tag="tat")
                        nc.sync.dma_start(
                            tat[:],
                            satt[:].rearrange("p (c w) -> p c w", c=NCHUNK, w=P),
                            transpose=True,
                        )
                        for c in range(NCHUNK):
                            nc.tensor.matmul(
                                acc[:],
                                tat[:, c, :],
                                inp2b[:, c * D : (c + 1) * D],
                                start=(c == 0), stop=(c == NCHUNK - 1),
                            )
                else:
                    grp = GRP_LAST if b == NBLK - 1 else GRP
                    GW = grp * P
                    GPP = max(1, PW // GW)  # groups per panel
                    for g in range(q * GPP, (q + 1) * GPP):
                        g0 = g * GW
                        sg = satt[:, g0 : g0 + GW]
                        nc.vector.tensor_scalar(
                            sg, num_t[b][:, g0 : g0 + GW], delta[:], None, AL.mult
                        )
                        eng = nc.gpsimd if ST_ENG[(g * GW) // PW] == "p" else nc.vector
                        eng.tensor_tensor(sg, sg, adjb[:, g0 : g0 + GW], AL.add)
                        stage = psA.tile([P, GW], BF16)
                        for k in range(grp):
                            c = g * grp + k
                            nc.tensor.matmul(
                                stage[:, k * P : (k + 1) * P],
                                satt[:, c * P : (c + 1) * P],
                                ident[:],
                                is_transpose=True, start=True, stop=True,
                            )
                        att = attp.tile([P, GW], BF16)
                        if COPY_ENG[g % len(COPY_ENG)] == "s":
                            nc.scalar.copy(att[:], stage[:])
                        else:
                            nc.vector.tensor_copy(att[:], stage[:])
                        for k in range(grp):
                            c = g * grp + k
                            nc.tensor.matmul(
                                acc[:],
                                att[:, k * P : (k + 1) * P],
                                inp2b[:, c * D : (c + 1) * D],
                                start=(c == 0), stop=(c == NCHUNK - 1),
                            )
            def emit_back_tail(b):
                satt, adjb, delta, acc = back_state.pop(b)
                ot = outp.tile([P, D], F32)
                nc.vector.tensor_copy(ot[:], acc[:])
                nc.sync.dma_start(out[b * P : (b + 1) * P, :], ot[:])

            def prefetch_adj(b):
                nx = adjp.tile([P, M], BF16, name="adjb", tag="adjb")
                adjb_t[b] = nx
                for q in range(NP):
                    nc.gpsimd.dma_start(
                        nx[:, q * PW : (q + 1) * PW],
                        adj[b * P : (b + 1) * P, q * PW : (q + 1) * PW],
                    )

            # front(b) fully first (its DVE/ACT ops are ready early), then
            # back(b-1) whose head waits on exp(b-1,3) anyway
            for b in range(NBLK + 2):
                if 2 <= b + 2 < NBLK:
                    prefetch_adj(b + 2)
                for q in range(NP):
                    if b < NBLK:
                        emit_front_panel(b, q)
                        if b == 0:
                            _setup_hook(q)
                if 1 <= b <= NBLK:
                    emit_back_head(b - 1)
                    for q in range(NP):
                        emit_back_panel(b - 1, q)
                if b >= 2:
                    emit_back_tail(b - 2)

    nc.compile()
    return nc


def _get_nc():
    global _CACHED
    if _CACHED is None:
        _CACHED = _build_kernel()
    return _CACHED


def kernel(input1, input2, adj, a1, a2):
    global LAST_EXEC_NS
    nc = _get_nc()

    try:
        import ml_dtypes

        bf16 = ml_dtypes.bfloat16
    except Exception:  # pragma: no cover
        bf16 = np.float32

    a1bv = np.ascontiguousarray(
        np.broadcast_to(np.asarray(a1, np.float32).reshape(1, D), (P, D))
    )
    a2bv = np.ascontiguousarray(
        np.broadcast_to(np.asarray(a2, np.float32).reshape(1, D), (P, D))
    )
    ident = np.eye(P, dtype=bf16)

    input1 = np.ascontiguousarray(input1, dtype=np.float32)
    input2 = np.ascontiguousarray(input2, dtype=np.float32)
    adj = np.ascontiguousarray(adj, dtype=np.float32)

    in_maps = []
    for c in range(NCORES):
        r0, r1 = c * ROWS, (c + 1) * ROWS
        in_maps.append(
            {
                "input1": input1[r0:r1],
                "input2": input2,
                "adj": adj[r0:r1],
                "a1b": a1bv,
                "a2b": a2bv,
                "identd": ident,
            }
        )

    trace = bool(os.environ.get("GAT_TRACE"))
    res = run_bass_kernel_spmd(nc, in_maps, core_ids=list(range(NCORES)), trace=trace)
    LAST_EXEC_NS = res.exec_time_ns
    outs = [res.results[c]["out"] for c in range(NCORES)]
    return np.concatenate(outs, axis=0).astype(np.float32)
